# revision 2
# baseline (speedup 1.0000x reference)
"""Causal self-attention (RoPE, 16 heads, B=2 T=2048 C=1024) on 8 TRN2 cores.

v2: keeps the PE warm (junk warm-up MMs + no >3.4us PE idle), reorders
phases so all RoPE tails drain during the v projection, bf16 rope pipeline,
explicit PSUM bank choreography, proj with split copies/DMA queues.

Sharding: core = b*4 + g  (b = batch, g = head-group of 4 heads).
"""

import numpy as np

B = 2
T = 2048
C = 1024
N_HEAD = 16
HD = 64
HPC = 4           # heads per core
N_CORES = 8
ROPE_BASE = 10000.0
TS = 512          # qkv t-slice width
VW = 128          # v_ext per-head width: v (64) + ones column + zero pad to
                  # 128 so every y-matmul drives all 128 PE array columns —
                  # the HAM clock-gate otherwise treats M=65 matmuls as
                  # "idle" and halves the PE clock for the attention phase

DTCFG = "bbbb"    # (qkv, scores, y, proj): 'b' = bfloat16

_CACHE = {}


def _chunks512(off, end):
    out = []
    lo = off
    while lo < end:
        hi = min(end, (lo // 512 + 1) * 512)
        out.append((lo, hi))
        lo = hi
    return out


def _np_dt(ch):
    if ch == "b":
        import ml_dtypes
        return np.dtype(ml_dtypes.bfloat16)
    return np.dtype(np.float32)


def _build(t_len=T, dtcfg=None, debug=False):
    import concourse.tile as tile
    from concourse import bacc, mybir

    dtcfg = dtcfg or DTCFG
    assert dtcfg == "bbbb", "v2 kernel supports bf16 config only"
    F32 = mybir.dt.float32
    BF16 = mybir.dt.bfloat16
    D_QKV = D_S = D_Y = D_P = BF16

    n_ts = t_len // TS          # qkv t-slices (4)
    n_tt = t_len // 128         # 128-row t-tiles (16)
    n_j = t_len // 1024         # attention 1024-wide tq slices (2)

    nc = bacc.Bacc(None, target_bir_lowering=False, debug=False)
    with tile.TileContext(nc) as tc:
        with tc.tile_pool(name="dram", bufs=1, space="DRAM") as dram:
            xT = dram.tile([C, t_len], D_QKV, kind="ExternalInput")
            wqk = dram.tile([C, 8 * HD], D_QKV, kind="ExternalInput")
            wv = dram.tile([C, 4 * HD], D_QKV, kind="ExternalInput")
            wo = dram.tile([4 * HD, C], D_P, kind="ExternalInput")
            cost = dram.tile([128, t_len], D_S, kind="ExternalInput")
            ssin = dram.tile([128, t_len], D_S, kind="ExternalInput")
            utri = dram.tile([128, 128], D_Y, kind="ExternalInput")
            ident = dram.tile([128, 128], D_Y, kind="ExternalInput")
            ones4 = dram.tile([128, (t_len // 128) * HPC * (VW - HD)], D_Y,
                              kind="ExternalInput")
            out = dram.tile([t_len, C], D_P, kind="ExternalOutput")

            xT_c = xT.rearrange("(a p) t -> a p t", p=128)    # [8, 128, T]
            wqk_c = wqk.rearrange("(a p) m -> a p m", p=128)  # [8, 128, 512]
            wv_c = wv.rearrange("(a p) m -> a p m", p=128)    # [8, 128, 256]
            wo_c = wo.rearrange("(a p) m -> a p m", p=128)    # [2, 128, 1024]

            with (
                tc.tile_pool(name="persist", bufs=1) as persist,
                tc.tile_pool(name="qkT_pool", bufs=1) as qkT_pool,
            ):
                ltri_sb = persist.tile([128, 128], D_Y)
                ident_sb = persist.tile([128, 128], D_Y)
                cos_sb = persist.tile([128, t_len], D_S)
                ssin_sb = persist.tile([128, t_len], D_S)
                qkT = [qkT_pool.tile([128, t_len], D_S, name=f"qkT{m}")
                       for m in range(4)]
                vext_sb = persist.tile([128, n_tt * HPC * VW], D_Y)
                vext_v = vext_sb.rearrange("p (i h d) -> p i h d", i=n_tt, d=VW)
                yT = [persist.tile([128, t_len], D_P, name=f"yT{k}")
                      for k in range(2)]
                xT_sb = [persist.tile([128, t_len], D_QKV, name=f"xTsb{c}")
                         for c in range(8)]
                wqk_sb = [persist.tile([128, 8 * HD], D_QKV, name=f"wqk{c}")
                          for c in range(8)]
                wv_sb = [persist.tile([128, 4 * HD], D_QKV, name=f"wv{c}")
                         for c in range(8)]
                wo_sb = [persist.tile([128, C], D_P, name=f"wo{k}")
                         for k in range(2)]

                # ---- input DMAs (spread across queues) ----
                # ident first on sync: the warm-up matmuls need it ASAP.
                nc.sync.dma_start(out=ident_sb, in_=ident[:])
                nc.sync.dma_start(out=ltri_sb, in_=utri[:])
                # weights on the scalar HWDGE queue
                for c in range(8):
                    nc.scalar.dma_start(out=wv_sb[c], in_=wv_c[c])
                for c in range(8):
                    nc.scalar.dma_start(out=wqk_sb[c], in_=wqk_c[c])
                # x in quarter-slices, quarter-major so the first m-group can
                # start after ~1MB instead of ~4MB; split sync/gpsimd queues.
                # x goes FIRST on gpsimd so the PE is never table-starved.
                for q in range(4):
                    for c in range(8):
                        xq = nc.sync if c % 2 == 0 else nc.gpsimd
                        xq.dma_start(
                            out=xT_sb[c][:, q * TS:(q + 1) * TS],
                            in_=xT_c[c, :, q * TS:(q + 1) * TS])
                # rope tables on scalar after the weights (needed ~12us in)
                nc.scalar.dma_start(out=cos_sb, in_=cost[:])
                nc.scalar.dma_start(out=ssin_sb, in_=ssin[:])
                nc.gpsimd.dma_start(
                    out=vext_v[:, :, :, HD:],
                    in_=ones4[:].rearrange("p (i h o) -> p i h o",
                                           i=n_tt, o=VW - HD),
                )
                # wo only needed at proj time: last on the gpsimd queue
                for k in range(2):
                    nc.gpsimd.dma_start(out=wo_sb[k], in_=wo_c[k])

                # ---- PE warm-up: ~4.5us of junk matmuls so the HAM clock
                # gate reaches K=8/8 before the real work arrives. ----
                with tc.tile_pool(name="warm_ps", bufs=1,
                                  space="PSUM") as warm_ps:
                    wps = warm_ps.tile([128, 128], F32)
                    for w in range(34):
                        nc.tensor.matmul(out=wps[:], lhsT=ident_sb,
                                         rhs=ident_sb, start=True, stop=True)

                # ---------------- qkv phase ----------------
                # rope_pool stays open for the whole kernel: if its SBUF
                # range were recycled into the attention pools, the first
                # exp/px writes would WAR-wait on the last rope chain
                # (measured 8-14us PE bubble at the phase transition).
                rope_cm = tc.tile_pool(name="rope_pool", bufs=2)
                rope_pool = rope_cm.__enter__()
                with (
                    tc.tile_pool(name="qk_ps", bufs=4, space="PSUM") as qk_ps,
                    tc.tile_pool(name="v_ps", bufs=2, space="PSUM") as v_ps,
                ):
                    def rope(qkps, m, t0):
                        """RoPE a projected q/k PSUM tile into qkT[m] (bf16).
                        The PSUM->SBUF copy runs on VECTOR: it must not sit
                        on the scalar queue ahead of the attention exps (it
                        gates both PSUM-bank reuse and the first scores)."""
                        qksb = rope_pool.tile([128, TS], D_S, tag="qksb",
                                              name=f"qksb_{m}_{t0}")
                        nc.vector.tensor_copy(out=qksb, in_=qkps[:, :TS])
                        swap = rope_pool.tile([128, TS], D_S, tag="swap",
                                              name=f"swap_{m}_{t0}")
                        # swap DMAs on sync (x streaming is done by then);
                        # keeping them off the scalar queue keeps the rope
                        # PSUM->SBUF copies prompt (they gate PSUM reuse)
                        for hb in (0, 64):
                            nc.sync.dma_start(
                                out=swap[hb:hb + 32, :],
                                in_=qksb[hb + 32:hb + 64, :])
                            nc.sync.dma_start(
                                out=swap[hb + 32:hb + 64, :],
                                in_=qksb[hb:hb + 32, :])
                        tmp1 = rope_pool.tile([128, TS], D_S, tag="tmp1",
                                              name=f"tmp1_{m}_{t0}")
                        nc.vector.tensor_mul(tmp1, qksb,
                                             cos_sb[:, t0:t0 + TS])
                        tmp2 = rope_pool.tile([128, TS], D_S, tag="tmp2",
                                              name=f"tmp2_{m}_{t0}")
                        nc.vector.tensor_mul(tmp2, swap,
                                             ssin_sb[:, t0:t0 + TS])
                        nc.vector.tensor_add(qkT[m][:, t0:t0 + TS],
                                             tmp1, tmp2)

                    def v_block(i0):
                        """v projection (natural layout) for i-tiles
                        i0..i0+3 — 4-tile blocks interleave between the q/k
                        m-groups to keep the PE fed during x streaming and
                        to absorb rope-chain latency."""
                        for i in range(i0, i0 + 4):
                            vps = v_ps.tile([128, 4 * HD], F32, tag="vps",
                                            name=f"vps_{i}")
                            for c in range(8):
                                nc.tensor.matmul(
                                    out=vps[:],
                                    lhsT=xT_sb[c][:, i * 128:(i + 1) * 128],
                                    rhs=wv_sb[c][:],
                                    start=(c == 0), stop=(c == 7),
                                )
                            nc.vector.tensor_copy(
                                out=vext_v[:, i, :, :HD],
                                in_=vps.rearrange("p (h d) -> p h d", d=HD),
                            )

                    def m_group(m):
                        # ts-outer: each t-slice finishes its c-accumulation
                        # after 8 MMs so its rope chain starts immediately,
                        # and ts0 only needs the first x quarter.
                        for ts in range(n_ts):
                            qkps = qk_ps.tile([128, TS], F32, tag="qkps",
                                              name=f"qkps_{m}_{ts}")
                            for c in range(8):
                                nc.tensor.matmul(
                                    out=qkps[:],
                                    lhsT=wqk_sb[c][:, m * 128:(m + 1) * 128],
                                    rhs=xT_sb[c][:, ts * TS:(ts + 1) * TS],
                                    start=(c == 0), stop=(c == 7),
                                )
                            rope(qkps, m, ts * TS)

                    # k01 q01 k23 q23 with v blocks between
                    v_block(0)
                    m_group(2)
                    v_block(4)
                    m_group(0)
                    v_block(8)
                    m_group(3)
                    v_block(12)
                    m_group(1)

                # ---------------- attention + norm ----------------
                # PSUM: yps pool opens first (banks 4-7, freed by v_ps),
                # sps pool second (banks 0-3, freed by qk_ps -- all rope
                # reads drained during the v phase).
                with (
                    tc.tile_pool(name="yps_pool", bufs=2,
                                 space="PSUM") as yps_pool,
                    tc.tile_pool(name="sps_pool", bufs=2,
                                 space="PSUM") as sps_pool,
                    tc.tile_pool(name="p_pool", bufs=8) as p_pool,
                    tc.tile_pool(name="n_pool", bufs=2) as n_pool,
                ):
                    def norm(yps_t, h, j):
                        base = 1024 * j
                        hoff = 64 * (h % 2)
                        ycp = n_pool.tile([65, 1024], F32, tag="ycp",
                                          name=f"ycp_{h}_{j}")
                        nc.vector.tensor_copy(out=ycp, in_=yps_t[0:65, :])
                        strip = n_pool.tile([8, 128], F32, tag="strip",
                                            name=f"strip_{h}_{j}")
                        nc.sync.dma_start(
                            out=strip,
                            in_=ycp[64:65, :].rearrange(
                                "p (a b) -> p a b", b=128))
                        rstrip = n_pool.tile([8, 128], F32, tag="rstrip",
                                             name=f"rstrip_{h}_{j}")
                        nc.vector.reciprocal_approx_fast(out=rstrip,
                                                         in_=strip)
                        rrow = n_pool.tile([1, 1024], F32, tag="rrow",
                                           name=f"rrow_{h}_{j}")
                        nc.sync.dma_start(
                            out=rrow.rearrange("p (a b) -> p a b", b=128),
                            in_=rstrip)
                        bcast = n_pool.tile([64, 1024], F32, tag="bcast",
                                            name=f"bcast_{h}_{j}")
                        nc.gpsimd.partition_broadcast(bcast[:], rrow[:])
                        nout = n_pool.tile([64, 1024], D_P, tag="nout",
                                           name=f"nout_{h}_{j}")
                        nc.vector.tensor_mul(nout, ycp[:64, :], bcast)
                        nc.sync.dma_start(
                            out=yT[h // 2][hoff:hoff + 64, base:base + 1024],
                            in_=nout,
                        )

                    for hp in range(2):
                        qtile, ktile = qkT[hp], qkT[2 + hp]
                        heads = (2 * hp, 2 * hp + 1)
                        for j in range(n_j):
                            base = 1024 * j
                            n_i = 8 * j + 8
                            yps = {h: yps_pool.tile([VW, 1024], F32,
                                                    tag="yps",
                                                    name=f"yps_{h}_{j}")
                                   for h in heads}
                            pend = {h: [] for h in heads}

                            def emit_s(h, i):
                                hoff = 64 * (h % 2)
                                c0 = max(base, 128 * i)
                                off = c0 - base
                                diag = i >= 8 * j
                                ch = _chunks512(off, 1024)
                                sx = sps_pool.tile([128, 1024], F32,
                                                   tag="sps",
                                                   name=f"sps_{h}_{j}_{i}")
                                for (lo, hi) in ch:
                                    # the first chunk holds the causal
                                    # diagonal block: keep its accumulation
                                    # group open for the additive mask MM
                                    is_diag_chunk = diag and lo == off
                                    nc.tensor.matmul(
                                        out=sx[:, lo:hi],
                                        lhsT=ktile[hoff:hoff + 64,
                                                   128 * i:128 * (i + 1)],
                                        rhs=qtile[hoff:hoff + 64,
                                                  base + lo:base + hi],
                                        start=True,
                                        stop=not is_diag_chunk,
                                    )
                                    if is_diag_chunk:
                                        # sx[p, off+q] += -1e4 for key p >
                                        # query q: masked exp underflows to 0
                                        nc.tensor.matmul(
                                            out=sx[:, off:off + 128],
                                            lhsT=ident_sb,
                                            rhs=ltri_sb,
                                            start=False, stop=True,
                                        )
                                px = p_pool.tile([128, 1024], D_Y, tag="psb",
                                                 name=f"psb_{h}_{j}_{i}")
                                nc.scalar.activation(
                                    out=px[:, off:], in_=sx[:, off:],
                                    func=mybir.ActivationFunctionType.Exp,
                                )
                                pend[h].append((i, px, ch))

                            def emit_y(h):
                                i, px, ch = pend[h].pop(0)
                                for (lo, hi) in reversed(ch):
                                    stop_i = 8 * j + (3 if lo < 512 else 7)
                                    base_v = (i * HPC + h) * VW
                                    nc.tensor.matmul(
                                        out=yps[h][:, lo:hi],
                                        lhsT=vext_sb[:, base_v:base_v + VW],
                                        rhs=px[:, lo:hi],
                                        start=(i == 0), stop=(i == stop_i),
                                    )

                            for h in heads:
                                emit_s(h, 0)
                            for i in range(1, n_i):
                                for h in heads:
                                    emit_s(h, i)
                                for h in heads:
                                    emit_y(h)
                            for h in heads:
                                emit_y(h)
                            for h in heads:
                                norm(yps[h], h, j)

                # ---------------- output projection ----------------
                with (
                    tc.tile_pool(name="osb_pool", bufs=4) as osb_pool,
                    tc.tile_pool(name="o_ps_pool", bufs=4,
                                 space="PSUM") as o_ps_pool,
                ):
                    if True:
                        for tt in range(n_tt):
                            ops = [o_ps_pool.tile([128, 512], F32, tag="ops",
                                                  name=f"ops_{tt}_{cs}")
                                   for cs in range(2)]
                            for k in range(2):
                                for cs in range(2):
                                    nc.tensor.matmul(
                                        out=ops[cs][:],
                                        lhsT=yT[k][:, tt * 128:(tt + 1) * 128],
                                        rhs=wo_sb[k][:, cs * 512:(cs + 1) * 512],
                                        start=(k == 0), stop=(k == 1),
                                    )
                            for cs in range(2):
                                osb = osb_pool.tile([128, 512], D_P,
                                                    tag="osb",
                                                    name=f"osb_{tt}_{cs}")
                                # split the PSUM->SBUF copies between scalar
                                # (idle after exp) and vector
                                if (tt + cs) % 2 == 0:
                                    nc.scalar.copy(out=osb, in_=ops[cs][:])
                                else:
                                    nc.vector.tensor_copy(out=osb,
                                                          in_=ops[cs][:])
                                dq = nc.sync if cs == 0 else nc.gpsimd
                                dq.dma_start(
                                    out=out[tt * 128:(tt + 1) * 128,
                                            cs * 512:(cs + 1) * 512],
                                    in_=osb,
                                )
                rope_cm.__exit__(None, None, None)
    nc.compile()
    names = dict(
        xT=xT.name, wqk=wqk.name, wv=wv.name, wo=wo.name,
        cost=cost.name, ssin=ssin.name, utri=utri.name, ident=ident.name,
        ones4=ones4.name, out=out.name,
    )
    return nc, names


# Head-dim permutation: evens first, odds last — turns the interleaved
# rotate-half pair swap into a contiguous 32-row block swap on device.
PERM = np.concatenate([np.arange(0, HD, 2), np.arange(1, HD, 2)])


def _host_constants(t_len=T, dtcfg=None):
    dtcfg = dtcfg or DTCFG
    inv_freq = 1.0 / (ROPE_BASE ** (np.arange(0, HD, 2, dtype=np.float64) / HD))
    t = np.arange(t_len, dtype=np.float64)
    freqs = np.outer(t, inv_freq)                      # [T, 32]
    emb = np.concatenate([freqs, freqs], axis=-1)      # [T, 64]
    cosT = np.cos(emb).T.astype(np.float32)            # [64, T]
    sinT = np.sin(emb).T.astype(np.float32)
    sgn = np.where(np.arange(HD) % 2 == 0, -1.0, 1.0).astype(np.float32)
    ssinT = sinT * sgn[:, None]
    cosP, ssinP = cosT[PERM], ssinT[PERM]
    d_s = _np_dt(dtcfg[1])
    cos128 = np.vstack([cosP, cosP]).astype(d_s)       # [128, T] bf16
    ssin128 = np.vstack([ssinP, ssinP]).astype(d_s)
    d_y = _np_dt(dtcfg[2])
    # strict-lower-triangular additive mask: exp(score - 1e4) underflows to 0
    ltri = (np.tril(np.full((128, 128), -1.0e4, dtype=np.float32), -1)
            ).astype(d_y)
    ident = np.eye(128, dtype=np.float32).astype(d_y)
    # per head-tile: col 64 = ones (softmax denominator), cols 65-127 = zero
    o4 = np.zeros((128, (t_len // 128) * HPC, VW - HD), dtype=np.float32)
    o4[:, :, 0] = 1.0
    ones4 = o4.reshape(128, -1).astype(d_y)
    return cos128, ssin128, ltri, ident, ones4


def _perm_heads(w):
    Cdim = w.shape[0]
    return w.reshape(Cdim, HPC, HD)[:, :, PERM].reshape(Cdim, HPC * HD)


def _core_inputs(x, w_attn, w_proj, t_len=T, dtcfg=None):
    dtcfg = dtcfg or DTCFG
    d_qkv, d_p = _np_dt(dtcfg[0]), _np_dt(dtcfg[3])
    cos128, ssin128, ltri, ident, ones4 = _host_constants(t_len, dtcfg)
    per_core = []
    for core in range(N_CORES):
        b, g = divmod(core, 4)
        h0 = g * HPC * HD
        wq = _perm_heads(w_attn[:, h0:h0 + HPC * HD])
        wk = _perm_heads(w_attn[:, C + h0:C + h0 + HPC * HD]
                         * np.float32(1.0 / np.sqrt(HD)))
        wvs = w_attn[:, 2 * C + h0:2 * C + h0 + HPC * HD]
        per_core.append(dict(
            xT=np.ascontiguousarray(x[b].T).astype(d_qkv),
            wqk=np.ascontiguousarray(np.concatenate([wq, wk], axis=1)).astype(d_qkv),
            wv=np.ascontiguousarray(wvs).astype(d_qkv),
            wo=np.ascontiguousarray(w_proj[h0:h0 + HPC * HD, :]).astype(d_p),
            cost=cos128, ssin=ssin128, utri=ltri, ident=ident, ones4=ones4,
        ))
    return per_core


def kernel(x, w_attn, w_proj):
    from concourse.bass_utils import run_bass_kernel_spmd

    x = np.asarray(x, dtype=np.float32)
    w_attn = np.asarray(w_attn, dtype=np.float32)
    w_proj = np.asarray(w_proj, dtype=np.float32)

    if "nc" not in _CACHE:
        _CACHE["nc"], _CACHE["names"] = _build(T)
    nc, names = _CACHE["nc"], _CACHE["names"]

    per_core = _core_inputs(x, w_attn, w_proj, T)
    in_maps = [{names[k]: v for k, v in m.items()} for m in per_core]
    r = run_bass_kernel_spmd(nc, in_maps, core_ids=list(range(N_CORES)))

    full = np.zeros((B, T, C), dtype=np.float64)
    for core in range(N_CORES):
        full[core // 4] += r.results[core][names["out"]].astype(np.float64)
    return full.astype(np.float32)


# revision 3
# speedup vs baseline: 1.0796x; 1.0796x over previous
"""Causal self-attention (RoPE, 16 heads, B=2 T=2048 C=1024) on 8 TRN2 cores.

v2: keeps the PE warm (junk warm-up MMs + no >3.4us PE idle), reorders
phases so all RoPE tails drain during the v projection, bf16 rope pipeline,
explicit PSUM bank choreography, proj with split copies/DMA queues.

Sharding: core = b*4 + g  (b = batch, g = head-group of 4 heads).
"""

import numpy as np

B = 2
T = 2048
C = 1024
N_HEAD = 16
HD = 64
HPC = 4           # heads per core
N_CORES = 8
ROPE_BASE = 10000.0
TS = 512          # qkv t-slice width
VW = 128          # v_ext per-head width: v (64) + ones column + zero pad to
                  # 128 so every y-matmul drives all 128 PE array columns —
                  # the HAM clock-gate otherwise treats M=65 matmuls as
                  # "idle" and halves the PE clock for the attention phase

DTCFG = "bbbb"    # (qkv, scores, y, proj): 'b' = bfloat16

_CACHE = {}


def _chunks512(off, end):
    out = []
    lo = off
    while lo < end:
        hi = min(end, (lo // 512 + 1) * 512)
        out.append((lo, hi))
        lo = hi
    return out


def _np_dt(ch):
    if ch == "b":
        import ml_dtypes
        return np.dtype(ml_dtypes.bfloat16)
    return np.dtype(np.float32)


def _build(t_len=T, dtcfg=None, debug=False):
    import concourse.tile as tile
    from concourse import bacc, mybir

    dtcfg = dtcfg or DTCFG
    assert dtcfg == "bbbb", "v2 kernel supports bf16 config only"
    F32 = mybir.dt.float32
    BF16 = mybir.dt.bfloat16
    D_QKV = D_S = D_Y = D_P = BF16

    n_ts = t_len // TS          # qkv t-slices (4)
    n_tt = t_len // 128         # 128-row t-tiles (16)
    n_j = t_len // 1024         # attention 1024-wide tq slices (2)

    nc = bacc.Bacc(None, target_bir_lowering=False, debug=False)
    with tile.TileContext(nc) as tc:
        with tc.tile_pool(name="dram", bufs=1, space="DRAM") as dram:
            xT = dram.tile([C, t_len], D_QKV, kind="ExternalInput")
            wqk = dram.tile([C, 8 * HD], D_QKV, kind="ExternalInput")
            wv = dram.tile([C, 4 * HD], D_QKV, kind="ExternalInput")
            wo = dram.tile([4 * HD, C], D_P, kind="ExternalInput")
            cost = dram.tile([128, t_len], D_S, kind="ExternalInput")
            ssin = dram.tile([128, t_len], D_S, kind="ExternalInput")
            utri = dram.tile([128, 128], D_Y, kind="ExternalInput")
            ident = dram.tile([128, 128], D_Y, kind="ExternalInput")
            ones4 = dram.tile([128, (t_len // 128) * HPC * (VW - HD)], D_Y,
                              kind="ExternalInput")
            out = dram.tile([t_len, C], D_P, kind="ExternalOutput")

            xT_c = xT.rearrange("(a p) t -> a p t", p=128)    # [8, 128, T]
            wqk_c = wqk.rearrange("(a p) m -> a p m", p=128)  # [8, 128, 512]
            wv_c = wv.rearrange("(a p) m -> a p m", p=128)    # [8, 128, 256]
            wo_c = wo.rearrange("(a p) m -> a p m", p=128)    # [2, 128, 1024]

            with (
                tc.tile_pool(name="persist", bufs=1) as persist,
                tc.tile_pool(name="qkT_pool", bufs=1) as qkT_pool,
            ):
                ltri_sb = persist.tile([128, 128], D_Y)
                ident_sb = persist.tile([128, 128], D_Y)
                cos_sb = persist.tile([128, t_len], D_S)
                ssin_sb = persist.tile([128, t_len], D_S)
                qkT = [qkT_pool.tile([128, t_len], D_S, name=f"qkT{m}")
                       for m in range(4)]
                vext_sb = persist.tile([128, n_tt * HPC * VW], D_Y)
                vext_v = vext_sb.rearrange("p (i h d) -> p i h d", i=n_tt, d=VW)
                yT = [persist.tile([128, t_len], D_P, name=f"yT{k}")
                      for k in range(2)]
                xT_sb = [persist.tile([128, t_len], D_QKV, name=f"xTsb{c}")
                         for c in range(8)]
                wqk_sb = [persist.tile([128, 8 * HD], D_QKV, name=f"wqk{c}")
                          for c in range(8)]
                wv_sb = [persist.tile([128, 4 * HD], D_QKV, name=f"wv{c}")
                         for c in range(8)]
                wo_sb = [persist.tile([128, C], D_P, name=f"wo{k}")
                         for k in range(2)]

                # ---- input DMAs (spread across queues) ----
                # ident first on sync: the warm-up matmuls need it ASAP.
                nc.sync.dma_start(out=ident_sb, in_=ident[:])
                nc.sync.dma_start(out=ltri_sb, in_=utri[:])
                # weights on the scalar HWDGE queue
                for c in range(8):
                    nc.scalar.dma_start(out=wv_sb[c], in_=wv_c[c])
                for c in range(8):
                    nc.scalar.dma_start(out=wqk_sb[c], in_=wqk_c[c])
                # x in quarter-slices, quarter-major so the first m-group can
                # start after ~1MB instead of ~4MB; split sync/gpsimd queues.
                # x goes FIRST on gpsimd so the PE is never table-starved.
                for q in range(4):
                    for c in range(8):
                        xq = nc.sync if c % 2 == 0 else nc.gpsimd
                        xq.dma_start(
                            out=xT_sb[c][:, q * TS:(q + 1) * TS],
                            in_=xT_c[c, :, q * TS:(q + 1) * TS])
                # rope tables on scalar after the weights (needed ~12us in)
                nc.scalar.dma_start(out=cos_sb, in_=cost[:])
                nc.scalar.dma_start(out=ssin_sb, in_=ssin[:])
                nc.gpsimd.dma_start(
                    out=vext_v[:, :, :, HD:],
                    in_=ones4[:].rearrange("p (i h o) -> p i h o",
                                           i=n_tt, o=VW - HD),
                )
                # wo only needed at proj time: last on the gpsimd queue
                for k in range(2):
                    nc.gpsimd.dma_start(out=wo_sb[k], in_=wo_c[k])

                # ---- PE warm-up: ~4.5us of junk matmuls so the HAM clock
                # gate reaches K=8/8 before the real work arrives. ----
                with tc.tile_pool(name="warm_ps", bufs=1,
                                  space="PSUM") as warm_ps:
                    wps = warm_ps.tile([128, 128], F32)
                    for w in range(34):
                        nc.tensor.matmul(out=wps[:], lhsT=ident_sb,
                                         rhs=ident_sb, start=True, stop=True)

                # ---------------- qkv phase ----------------
                # rope_pool stays open for the whole kernel: if its SBUF
                # range were recycled into the attention pools, the first
                # exp/px writes would WAR-wait on the last rope chain
                # (measured 8-14us PE bubble at the phase transition).
                rope_cm = tc.tile_pool(name="rope_pool", bufs=2)
                rope_pool = rope_cm.__enter__()
                with (
                    tc.tile_pool(name="qk_ps", bufs=4, space="PSUM") as qk_ps,
                    tc.tile_pool(name="v_ps", bufs=2, space="PSUM") as v_ps,
                ):
                    def rope(qkps, m, t0):
                        """RoPE a projected q/k PSUM tile into qkT[m] (bf16).
                        The PSUM->SBUF copy runs on VECTOR: it must not sit
                        on the scalar queue ahead of the attention exps (it
                        gates both PSUM-bank reuse and the first scores)."""
                        qksb = rope_pool.tile([128, TS], D_S, tag="qksb",
                                              name=f"qksb_{m}_{t0}")
                        nc.vector.tensor_copy(out=qksb, in_=qkps[:, :TS])
                        swap = rope_pool.tile([128, TS], D_S, tag="swap",
                                              name=f"swap_{m}_{t0}")
                        # swap DMAs on sync (x streaming is done by then);
                        # keeping them off the scalar queue keeps the rope
                        # PSUM->SBUF copies prompt (they gate PSUM reuse)
                        for hb in (0, 64):
                            nc.sync.dma_start(
                                out=swap[hb:hb + 32, :],
                                in_=qksb[hb + 32:hb + 64, :])
                            nc.sync.dma_start(
                                out=swap[hb + 32:hb + 64, :],
                                in_=qksb[hb:hb + 32, :])
                        tmp1 = rope_pool.tile([128, TS], D_S, tag="tmp1",
                                              name=f"tmp1_{m}_{t0}")
                        nc.vector.tensor_mul(tmp1, qksb,
                                             cos_sb[:, t0:t0 + TS])
                        tmp2 = rope_pool.tile([128, TS], D_S, tag="tmp2",
                                              name=f"tmp2_{m}_{t0}")
                        nc.vector.tensor_mul(tmp2, swap,
                                             ssin_sb[:, t0:t0 + TS])
                        nc.vector.tensor_add(qkT[m][:, t0:t0 + TS],
                                             tmp1, tmp2)

                    def v_block(i0):
                        """v projection (natural layout) for i-tiles
                        i0..i0+3 — 4-tile blocks interleave between the q/k
                        m-groups to keep the PE fed during x streaming and
                        to absorb rope-chain latency."""
                        for i in range(i0, i0 + 4):
                            vps = v_ps.tile([128, 4 * HD], F32, tag="vps",
                                            name=f"vps_{i}")
                            for c in range(8):
                                nc.tensor.matmul(
                                    out=vps[:],
                                    lhsT=xT_sb[c][:, i * 128:(i + 1) * 128],
                                    rhs=wv_sb[c][:],
                                    start=(c == 0), stop=(c == 7),
                                )
                            nc.vector.tensor_copy(
                                out=vext_v[:, i, :, :HD],
                                in_=vps.rearrange("p (h d) -> p h d", d=HD),
                            )

                    def m_group(m):
                        # ts-outer: each t-slice finishes its c-accumulation
                        # after 8 MMs so its rope chain starts immediately,
                        # and ts0 only needs the first x quarter.
                        for ts in range(n_ts):
                            qkps = qk_ps.tile([128, TS], F32, tag="qkps",
                                              name=f"qkps_{m}_{ts}")
                            for c in range(8):
                                nc.tensor.matmul(
                                    out=qkps[:],
                                    lhsT=wqk_sb[c][:, m * 128:(m + 1) * 128],
                                    rhs=xT_sb[c][:, ts * TS:(ts + 1) * TS],
                                    start=(c == 0), stop=(c == 7),
                                )
                            rope(qkps, m, ts * TS)

                    # k01 q01 k23 q23 with v blocks between
                    v_block(0)
                    m_group(2)
                    v_block(4)
                    m_group(0)
                    v_block(8)
                    m_group(3)
                    v_block(12)
                    m_group(1)

                # ---------------- attention + norm ----------------
                # PSUM: yps pool opens first (banks 4-7, freed by v_ps),
                # sps pool second (banks 0-3, freed by qk_ps -- all rope
                # reads drained during the v phase).
                with (
                    tc.tile_pool(name="yps_pool", bufs=2,
                                 space="PSUM") as yps_pool,
                    tc.tile_pool(name="sps_pool", bufs=2,
                                 space="PSUM") as sps_pool,
                    tc.tile_pool(name="p_pool", bufs=8) as p_pool,
                    tc.tile_pool(name="n_pool", bufs=2) as n_pool,
                ):
                    def norm(yps_t, h, j):
                        base = 1024 * j
                        hoff = 64 * (h % 2)
                        ycp = n_pool.tile([65, 1024], F32, tag="ycp",
                                          name=f"ycp_{h}_{j}")
                        nc.vector.tensor_copy(out=ycp, in_=yps_t[0:65, :])
                        strip = n_pool.tile([8, 128], F32, tag="strip",
                                            name=f"strip_{h}_{j}")
                        nc.sync.dma_start(
                            out=strip,
                            in_=ycp[64:65, :].rearrange(
                                "p (a b) -> p a b", b=128))
                        rstrip = n_pool.tile([8, 128], F32, tag="rstrip",
                                             name=f"rstrip_{h}_{j}")
                        nc.vector.reciprocal_approx_fast(out=rstrip,
                                                         in_=strip)
                        rrow = n_pool.tile([1, 1024], F32, tag="rrow",
                                           name=f"rrow_{h}_{j}")
                        nc.sync.dma_start(
                            out=rrow.rearrange("p (a b) -> p a b", b=128),
                            in_=rstrip)
                        bcast = n_pool.tile([64, 1024], F32, tag="bcast",
                                            name=f"bcast_{h}_{j}")
                        nc.gpsimd.partition_broadcast(bcast[:], rrow[:])
                        nout = n_pool.tile([64, 1024], D_P, tag="nout",
                                           name=f"nout_{h}_{j}")
                        nc.vector.tensor_mul(nout, ycp[:64, :], bcast)
                        nc.sync.dma_start(
                            out=yT[h // 2][hoff:hoff + 64, base:base + 1024],
                            in_=nout,
                        )

                    # deferred norm thunks: each block's norms are emitted
                    # AFTER the next block's first scores, so the PE never
                    # sees a boundary bubble (a ~0.7us gap there re-throttles
                    # the HAM clock gate for the rest of the attention phase)
                    deferred = []
                    for hp in range(2):
                        qtile, ktile = qkT[hp], qkT[2 + hp]
                        heads = (2 * hp, 2 * hp + 1)
                        for j in range(n_j):
                            base = 1024 * j
                            n_i = 8 * j + 8
                            yps = {h: yps_pool.tile([VW, 1024], F32,
                                                    tag="yps",
                                                    name=f"yps_{h}_{j}")
                                   for h in heads}
                            pend = {h: [] for h in heads}

                            def emit_s(h, i):
                                hoff = 64 * (h % 2)
                                c0 = max(base, 128 * i)
                                off = c0 - base
                                diag = i >= 8 * j
                                ch = _chunks512(off, 1024)
                                sx = sps_pool.tile([128, 1024], F32,
                                                   tag="sps",
                                                   name=f"sps_{h}_{j}_{i}")
                                for (lo, hi) in ch:
                                    # the first chunk holds the causal
                                    # diagonal block: keep its accumulation
                                    # group open for the additive mask MM
                                    is_diag_chunk = diag and lo == off
                                    nc.tensor.matmul(
                                        out=sx[:, lo:hi],
                                        lhsT=ktile[hoff:hoff + 64,
                                                   128 * i:128 * (i + 1)],
                                        rhs=qtile[hoff:hoff + 64,
                                                  base + lo:base + hi],
                                        start=True,
                                        stop=not is_diag_chunk,
                                    )
                                    if is_diag_chunk:
                                        # sx[p, off+q] += -1e4 for key p >
                                        # query q: masked exp underflows to 0
                                        nc.tensor.matmul(
                                            out=sx[:, off:off + 128],
                                            lhsT=ident_sb,
                                            rhs=ltri_sb,
                                            start=False, stop=True,
                                        )
                                px = p_pool.tile([128, 1024], D_Y, tag="psb",
                                                 name=f"psb_{h}_{j}_{i}")
                                nc.scalar.activation(
                                    out=px[:, off:], in_=sx[:, off:],
                                    func=mybir.ActivationFunctionType.Exp,
                                )
                                pend[h].append((i, px, ch))

                            def emit_y(h):
                                i, px, ch = pend[h].pop(0)
                                for (lo, hi) in reversed(ch):
                                    stop_i = 8 * j + (3 if lo < 512 else 7)
                                    base_v = (i * HPC + h) * VW
                                    nc.tensor.matmul(
                                        out=yps[h][:, lo:hi],
                                        lhsT=vext_sb[:, base_v:base_v + VW],
                                        rhs=px[:, lo:hi],
                                        start=(i == 0), stop=(i == stop_i),
                                    )

                            for h in heads:
                                emit_s(h, 0)
                            for t in deferred:
                                t()
                            deferred = []
                            for i in range(1, n_i):
                                for h in heads:
                                    emit_s(h, i)
                                for h in heads:
                                    emit_y(h)
                            for h in heads:
                                emit_y(h)
                            deferred = [
                                (lambda yt=yps[h], hh=h, jj=j:
                                 norm(yt, hh, jj))
                                for h in heads
                            ]
                    for t in deferred:
                        t()

                # ---------------- output projection ----------------
                with (
                    tc.tile_pool(name="osb_pool", bufs=4) as osb_pool,
                    tc.tile_pool(name="o_ps_pool", bufs=4,
                                 space="PSUM") as o_ps_pool,
                ):
                    if True:
                        for tt in range(n_tt):
                            ops = [o_ps_pool.tile([128, 512], F32, tag="ops",
                                                  name=f"ops_{tt}_{cs}")
                                   for cs in range(2)]
                            for k in range(2):
                                for cs in range(2):
                                    nc.tensor.matmul(
                                        out=ops[cs][:],
                                        lhsT=yT[k][:, tt * 128:(tt + 1) * 128],
                                        rhs=wo_sb[k][:, cs * 512:(cs + 1) * 512],
                                        start=(k == 0), stop=(k == 1),
                                    )
                            for cs in range(2):
                                osb = osb_pool.tile([128, 512], D_P,
                                                    tag="osb",
                                                    name=f"osb_{tt}_{cs}")
                                # split the PSUM->SBUF copies between scalar
                                # (idle after exp) and vector
                                if (tt + cs) % 2 == 0:
                                    nc.scalar.copy(out=osb, in_=ops[cs][:])
                                else:
                                    nc.vector.tensor_copy(out=osb,
                                                          in_=ops[cs][:])
                                dq = nc.sync if cs == 0 else nc.gpsimd
                                dq.dma_start(
                                    out=out[tt * 128:(tt + 1) * 128,
                                            cs * 512:(cs + 1) * 512],
                                    in_=osb,
                                )
                rope_cm.__exit__(None, None, None)
    nc.compile()
    names = dict(
        xT=xT.name, wqk=wqk.name, wv=wv.name, wo=wo.name,
        cost=cost.name, ssin=ssin.name, utri=utri.name, ident=ident.name,
        ones4=ones4.name, out=out.name,
    )
    return nc, names


# Head-dim permutation: evens first, odds last — turns the interleaved
# rotate-half pair swap into a contiguous 32-row block swap on device.
PERM = np.concatenate([np.arange(0, HD, 2), np.arange(1, HD, 2)])


def _host_constants(t_len=T, dtcfg=None):
    dtcfg = dtcfg or DTCFG
    inv_freq = 1.0 / (ROPE_BASE ** (np.arange(0, HD, 2, dtype=np.float64) / HD))
    t = np.arange(t_len, dtype=np.float64)
    freqs = np.outer(t, inv_freq)                      # [T, 32]
    emb = np.concatenate([freqs, freqs], axis=-1)      # [T, 64]
    cosT = np.cos(emb).T.astype(np.float32)            # [64, T]
    sinT = np.sin(emb).T.astype(np.float32)
    sgn = np.where(np.arange(HD) % 2 == 0, -1.0, 1.0).astype(np.float32)
    ssinT = sinT * sgn[:, None]
    cosP, ssinP = cosT[PERM], ssinT[PERM]
    d_s = _np_dt(dtcfg[1])
    cos128 = np.vstack([cosP, cosP]).astype(d_s)       # [128, T] bf16
    ssin128 = np.vstack([ssinP, ssinP]).astype(d_s)
    d_y = _np_dt(dtcfg[2])
    # strict-lower-triangular additive mask: exp(score - 1e4) underflows to 0
    ltri = (np.tril(np.full((128, 128), -1.0e4, dtype=np.float32), -1)
            ).astype(d_y)
    ident = np.eye(128, dtype=np.float32).astype(d_y)
    # per head-tile: col 64 = ones (softmax denominator), cols 65-127 = zero
    o4 = np.zeros((128, (t_len // 128) * HPC, VW - HD), dtype=np.float32)
    o4[:, :, 0] = 1.0
    ones4 = o4.reshape(128, -1).astype(d_y)
    return cos128, ssin128, ltri, ident, ones4


def _perm_heads(w):
    Cdim = w.shape[0]
    return w.reshape(Cdim, HPC, HD)[:, :, PERM].reshape(Cdim, HPC * HD)


def _core_inputs(x, w_attn, w_proj, t_len=T, dtcfg=None):
    dtcfg = dtcfg or DTCFG
    d_qkv, d_p = _np_dt(dtcfg[0]), _np_dt(dtcfg[3])
    cos128, ssin128, ltri, ident, ones4 = _host_constants(t_len, dtcfg)
    per_core = []
    for core in range(N_CORES):
        b, g = divmod(core, 4)
        h0 = g * HPC * HD
        wq = _perm_heads(w_attn[:, h0:h0 + HPC * HD])
        wk = _perm_heads(w_attn[:, C + h0:C + h0 + HPC * HD]
                         * np.float32(1.0 / np.sqrt(HD)))
        wvs = w_attn[:, 2 * C + h0:2 * C + h0 + HPC * HD]
        per_core.append(dict(
            xT=np.ascontiguousarray(x[b].T).astype(d_qkv),
            wqk=np.ascontiguousarray(np.concatenate([wq, wk], axis=1)).astype(d_qkv),
            wv=np.ascontiguousarray(wvs).astype(d_qkv),
            wo=np.ascontiguousarray(w_proj[h0:h0 + HPC * HD, :]).astype(d_p),
            cost=cos128, ssin=ssin128, utri=ltri, ident=ident, ones4=ones4,
        ))
    return per_core


def kernel(x, w_attn, w_proj):
    from concourse.bass_utils import run_bass_kernel_spmd

    x = np.asarray(x, dtype=np.float32)
    w_attn = np.asarray(w_attn, dtype=np.float32)
    w_proj = np.asarray(w_proj, dtype=np.float32)

    if "nc" not in _CACHE:
        _CACHE["nc"], _CACHE["names"] = _build(T)
    nc, names = _CACHE["nc"], _CACHE["names"]

    per_core = _core_inputs(x, w_attn, w_proj, T)
    in_maps = [{names[k]: v for k, v in m.items()} for m in per_core]
    r = run_bass_kernel_spmd(nc, in_maps, core_ids=list(range(N_CORES)))

    full = np.zeros((B, T, C), dtype=np.float64)
    for core in range(N_CORES):
        full[core // 4] += r.results[core][names["out"]].astype(np.float64)
    return full.astype(np.float32)


# revision 4
# speedup vs baseline: 1.2222x; 1.1321x over previous
"""Causal self-attention (RoPE, 16 heads, B=2 T=2048 C=1024) on 8 TRN2 cores.

v2: keeps the PE warm (junk warm-up MMs + no >3.4us PE idle), reorders
phases so all RoPE tails drain during the v projection, bf16 rope pipeline,
explicit PSUM bank choreography, proj with split copies/DMA queues.

Sharding: core = b*4 + g  (b = batch, g = head-group of 4 heads).
"""

import numpy as np

B = 2
T = 2048
C = 1024
N_HEAD = 16
HD = 64
HPC = 4           # heads per core
N_CORES = 8
ROPE_BASE = 10000.0
TS = 512          # qkv t-slice width
VW = 128          # v_ext per-head width: v (64) + ones column + zero pad to
                  # 128 so every y-matmul drives all 128 PE array columns —
                  # the HAM clock-gate otherwise treats M=65 matmuls as
                  # "idle" and halves the PE clock for the attention phase

DTCFG = "bbbb"    # (qkv, scores, y, proj): 'b' = bfloat16

_CACHE = {}


def _chunks512(off, end):
    out = []
    lo = off
    while lo < end:
        hi = min(end, (lo // 512 + 1) * 512)
        out.append((lo, hi))
        lo = hi
    return out


def _np_dt(ch):
    if ch == "b":
        import ml_dtypes
        return np.dtype(ml_dtypes.bfloat16)
    return np.dtype(np.float32)


def _build(t_len=T, dtcfg=None, debug=False):
    import concourse.tile as tile
    from concourse import bacc, mybir

    dtcfg = dtcfg or DTCFG
    assert dtcfg == "bbbb", "v2 kernel supports bf16 config only"
    F32 = mybir.dt.float32
    BF16 = mybir.dt.bfloat16
    D_QKV = D_S = D_Y = D_P = BF16

    n_ts = t_len // TS          # qkv t-slices (4)
    n_tt = t_len // 128         # 128-row t-tiles (16)
    n_j = t_len // 1024         # attention 1024-wide tq slices (2)

    nc = bacc.Bacc(None, target_bir_lowering=False, debug=False)
    with tile.TileContext(nc) as tc:
        with tc.tile_pool(name="dram", bufs=1, space="DRAM") as dram:
            xT = dram.tile([C, t_len], D_QKV, kind="ExternalInput")
            wqk = dram.tile([C, 8 * HD], D_QKV, kind="ExternalInput")
            wv = dram.tile([C, 4 * HD], D_QKV, kind="ExternalInput")
            wo = dram.tile([4 * HD, C], D_P, kind="ExternalInput")
            cost = dram.tile([128, t_len], D_S, kind="ExternalInput")
            ssin = dram.tile([128, t_len], D_S, kind="ExternalInput")
            utri = dram.tile([128, 128], D_Y, kind="ExternalInput")
            ident = dram.tile([128, 128], D_Y, kind="ExternalInput")
            ones4 = dram.tile([128, (t_len // 128) * HPC * (VW - HD)], D_Y,
                              kind="ExternalInput")
            out = dram.tile([t_len, C], D_P, kind="ExternalOutput")

            xT_c = xT.rearrange("(a p) t -> a p t", p=128)    # [8, 128, T]
            wqk_c = wqk.rearrange("(a p) m -> a p m", p=128)  # [8, 128, 512]
            wv_c = wv.rearrange("(a p) m -> a p m", p=128)    # [8, 128, 256]
            wo_c = wo.rearrange("(a p) m -> a p m", p=128)    # [2, 128, 1024]

            with (
                tc.tile_pool(name="persist", bufs=1) as persist,
                tc.tile_pool(name="qkT_pool", bufs=1) as qkT_pool,
            ):
                ltri_sb = persist.tile([128, 128], D_Y)
                ident_sb = persist.tile([128, 128], D_Y)
                cos_sb = persist.tile([128, t_len], D_S)
                ssin_sb = persist.tile([128, t_len], D_S)
                qkT = [qkT_pool.tile([128, t_len], D_S, name=f"qkT{m}")
                       for m in range(4)]
                vext_sb = persist.tile([128, n_tt * HPC * VW], D_Y)
                vext_v = vext_sb.rearrange("p (i h d) -> p i h d", i=n_tt, d=VW)
                yT = [persist.tile([128, t_len], D_P, name=f"yT{k}")
                      for k in range(2)]
                xT_sb = [persist.tile([128, t_len], D_QKV, name=f"xTsb{c}")
                         for c in range(8)]
                wqk_sb = [persist.tile([128, 8 * HD], D_QKV, name=f"wqk{c}")
                          for c in range(8)]
                wv_sb = [persist.tile([128, 4 * HD], D_QKV, name=f"wv{c}")
                         for c in range(8)]
                wo_sb = [persist.tile([128, C], D_P, name=f"wo{k}")
                         for k in range(2)]

                # ---- input DMAs (spread across queues) ----
                # ident first on sync: the warm-up matmuls need it ASAP.
                nc.sync.dma_start(out=ident_sb, in_=ident[:])
                nc.sync.dma_start(out=ltri_sb, in_=utri[:])
                # weights on the scalar HWDGE queue
                for c in range(8):
                    nc.scalar.dma_start(out=wv_sb[c], in_=wv_c[c])
                for c in range(8):
                    nc.scalar.dma_start(out=wqk_sb[c], in_=wqk_c[c])
                # x in quarter-slices, quarter-major so the first m-group can
                # start after ~1MB instead of ~4MB; split sync/gpsimd queues.
                # x goes FIRST on gpsimd so the PE is never table-starved.
                for q in range(4):
                    for c in range(8):
                        xq = nc.sync if c % 2 == 0 else nc.gpsimd
                        xq.dma_start(
                            out=xT_sb[c][:, q * TS:(q + 1) * TS],
                            in_=xT_c[c, :, q * TS:(q + 1) * TS])
                # rope tables on scalar after the weights (needed ~12us in)
                nc.scalar.dma_start(out=cos_sb, in_=cost[:])
                nc.scalar.dma_start(out=ssin_sb, in_=ssin[:])
                nc.gpsimd.dma_start(
                    out=vext_v[:, :, :, HD:],
                    in_=ones4[:].rearrange("p (i h o) -> p i h o",
                                           i=n_tt, o=VW - HD),
                )
                # wo only needed at proj time: last on the gpsimd queue
                for k in range(2):
                    nc.gpsimd.dma_start(out=wo_sb[k], in_=wo_c[k])

                # ---- PE warm-up: ~4.5us of junk matmuls so the HAM clock
                # gate reaches K=8/8 before the real work arrives. ----
                with tc.tile_pool(name="warm_ps", bufs=1,
                                  space="PSUM") as warm_ps:
                    wps = warm_ps.tile([128, 128], F32)
                    for w in range(34):
                        nc.tensor.matmul(out=wps[:], lhsT=ident_sb,
                                         rhs=ident_sb, start=True, stop=True)

                # ---------------- qkv phase ----------------
                # rope_pool stays open for the whole kernel: if its SBUF
                # range were recycled into the attention pools, the first
                # exp/px writes would WAR-wait on the last rope chain
                # (measured 8-14us PE bubble at the phase transition).
                rope_cm = tc.tile_pool(name="rope_pool", bufs=2)
                rope_pool = rope_cm.__enter__()
                with (
                    tc.tile_pool(name="qk_ps", bufs=4, space="PSUM") as qk_ps,
                    tc.tile_pool(name="v_ps", bufs=2, space="PSUM") as v_ps,
                ):
                    def rope(qkps, m, t0):
                        """RoPE a projected q/k PSUM tile into qkT[m] (bf16).
                        The PSUM->SBUF copy runs on VECTOR: it must not sit
                        on the scalar queue ahead of the attention exps (it
                        gates both PSUM-bank reuse and the first scores)."""
                        qksb = rope_pool.tile([128, TS], D_S, tag="qksb",
                                              name=f"qksb_{m}_{t0}")
                        nc.vector.tensor_copy(out=qksb, in_=qkps[:, :TS])
                        swap = rope_pool.tile([128, TS], D_S, tag="swap",
                                              name=f"swap_{m}_{t0}")
                        # swap DMAs on sync (x streaming is done by then);
                        # keeping them off the scalar queue keeps the rope
                        # PSUM->SBUF copies prompt (they gate PSUM reuse)
                        for hb in (0, 64):
                            nc.sync.dma_start(
                                out=swap[hb:hb + 32, :],
                                in_=qksb[hb + 32:hb + 64, :])
                            nc.sync.dma_start(
                                out=swap[hb + 32:hb + 64, :],
                                in_=qksb[hb:hb + 32, :])
                        tmp1 = rope_pool.tile([128, TS], D_S, tag="tmp1",
                                              name=f"tmp1_{m}_{t0}")
                        nc.vector.tensor_mul(tmp1, qksb,
                                             cos_sb[:, t0:t0 + TS])
                        tmp2 = rope_pool.tile([128, TS], D_S, tag="tmp2",
                                              name=f"tmp2_{m}_{t0}")
                        nc.vector.tensor_mul(tmp2, swap,
                                             ssin_sb[:, t0:t0 + TS])
                        nc.vector.tensor_add(qkT[m][:, t0:t0 + TS],
                                             tmp1, tmp2)

                    def v_block(i0):
                        """v projection (natural layout) for i-tiles
                        i0..i0+3 — 4-tile blocks interleave between the q/k
                        m-groups to keep the PE fed during x streaming and
                        to absorb rope-chain latency."""
                        for i in range(i0, i0 + 4):
                            vps = v_ps.tile([128, 4 * HD], F32, tag="vps",
                                            name=f"vps_{i}")
                            for c in range(8):
                                nc.tensor.matmul(
                                    out=vps[:],
                                    lhsT=xT_sb[c][:, i * 128:(i + 1) * 128],
                                    rhs=wv_sb[c][:],
                                    start=(c == 0), stop=(c == 7),
                                )
                            nc.vector.tensor_copy(
                                out=vext_v[:, i, :, :HD],
                                in_=vps.rearrange("p (h d) -> p h d", d=HD),
                            )

                    def m_group(m):
                        # ts-outer: each t-slice finishes its c-accumulation
                        # after 8 MMs so its rope chain starts immediately,
                        # and ts0 only needs the first x quarter.
                        for ts in range(n_ts):
                            qkps = qk_ps.tile([128, TS], F32, tag="qkps",
                                              name=f"qkps_{m}_{ts}")
                            for c in range(8):
                                nc.tensor.matmul(
                                    out=qkps[:],
                                    lhsT=wqk_sb[c][:, m * 128:(m + 1) * 128],
                                    rhs=xT_sb[c][:, ts * TS:(ts + 1) * TS],
                                    start=(c == 0), stop=(c == 7),
                                )
                            rope(qkps, m, ts * TS)

                    # k01 q01 k23 q23 with v blocks between
                    v_block(0)
                    m_group(2)
                    v_block(4)
                    m_group(0)
                    v_block(8)
                    m_group(3)
                    v_block(12)
                    m_group(1)
                    # seam bridge: ~2.5us of full-array junk MMs carries the
                    # HAM activity monitor across the first scores-only
                    # stretch of attention (weak K=64 activity) until the
                    # full-array y-matmuls join after the first exp —
                    # otherwise the clock halves for the whole phase
                    seam = v_ps.tile([128, 4 * HD], F32, tag="vps",
                                     name="seam_junk")
                    for w in range(24):
                        nc.tensor.matmul(out=seam[:], lhsT=ident_sb,
                                         rhs=xT_sb[0][:, 0:4 * HD],
                                         start=True, stop=True)

                # ---------------- attention + norm ----------------
                # PSUM: yps pool opens first (banks 4-7, freed by v_ps),
                # sps pool second (banks 0-3, freed by qk_ps -- all rope
                # reads drained during the v phase).
                with (
                    tc.tile_pool(name="yps_pool", bufs=2,
                                 space="PSUM") as yps_pool,
                    tc.tile_pool(name="sps_pool", bufs=2,
                                 space="PSUM") as sps_pool,
                    tc.tile_pool(name="p_pool", bufs=8) as p_pool,
                    tc.tile_pool(name="n_pool", bufs=2) as n_pool,
                ):
                    def norm(yps_t, h, j):
                        base = 1024 * j
                        hoff = 64 * (h % 2)
                        ycp = n_pool.tile([65, 1024], F32, tag="ycp",
                                          name=f"ycp_{h}_{j}")
                        nc.vector.tensor_copy(out=ycp, in_=yps_t[0:65, :])
                        strip = n_pool.tile([8, 128], F32, tag="strip",
                                            name=f"strip_{h}_{j}")
                        nc.sync.dma_start(
                            out=strip,
                            in_=ycp[64:65, :].rearrange(
                                "p (a b) -> p a b", b=128))
                        rstrip = n_pool.tile([8, 128], F32, tag="rstrip",
                                             name=f"rstrip_{h}_{j}")
                        nc.vector.reciprocal_approx_fast(out=rstrip,
                                                         in_=strip)
                        rrow = n_pool.tile([1, 1024], F32, tag="rrow",
                                           name=f"rrow_{h}_{j}")
                        nc.sync.dma_start(
                            out=rrow.rearrange("p (a b) -> p a b", b=128),
                            in_=rstrip)
                        bcast = n_pool.tile([64, 1024], F32, tag="bcast",
                                            name=f"bcast_{h}_{j}")
                        nc.gpsimd.partition_broadcast(bcast[:], rrow[:])
                        nout = n_pool.tile([64, 1024], D_P, tag="nout",
                                           name=f"nout_{h}_{j}")
                        nc.vector.tensor_mul(nout, ycp[:64, :], bcast)
                        nc.sync.dma_start(
                            out=yT[h // 2][hoff:hoff + 64, base:base + 1024],
                            in_=nout,
                        )

                    # deferred norm thunks: each block's norms are emitted
                    # AFTER the next block's first scores, so the PE never
                    # sees a boundary bubble (a ~0.7us gap there re-throttles
                    # the HAM clock gate for the rest of the attention phase)
                    deferred = []
                    for hp in range(2):
                        qtile, ktile = qkT[hp], qkT[2 + hp]
                        heads = (2 * hp, 2 * hp + 1)
                        for j in range(n_j):
                            base = 1024 * j
                            n_i = 8 * j + 8
                            yps = {h: yps_pool.tile([VW, 1024], F32,
                                                    tag="yps",
                                                    name=f"yps_{h}_{j}")
                                   for h in heads}
                            pend = {h: [] for h in heads}

                            def emit_s2(i):
                                """Scores for BOTH heads of the pair with
                                the rg0/rg64 chunk matmuls emitted
                                adjacently: they run concurrently on
                                disjoint PE row groups (the full-array mask
                                MMs would otherwise serialize the pair)."""
                                c0 = max(base, 128 * i)
                                off = c0 - base
                                diag = i >= 8 * j
                                ch = _chunks512(off, 1024)
                                sx = {h: sps_pool.tile(
                                        [128, 1024], F32, tag="sps",
                                        name=f"sps_{h}_{j}_{i}")
                                      for h in heads}
                                for (lo, hi) in ch:
                                    # the first chunk holds the causal
                                    # diagonal block: keep its accumulation
                                    # group open for the additive mask MM
                                    is_diag_chunk = diag and lo == off
                                    for h in heads:
                                        hoff = 64 * (h % 2)
                                        nc.tensor.matmul(
                                            out=sx[h][:, lo:hi],
                                            lhsT=ktile[hoff:hoff + 64,
                                                       128 * i:128 * (i + 1)],
                                            rhs=qtile[hoff:hoff + 64,
                                                      base + lo:base + hi],
                                            start=True,
                                            stop=not is_diag_chunk,
                                        )
                                if diag:
                                    for h in heads:
                                        # sx[p, off+q] += -1e4 for key p >
                                        # query q: exp underflows to 0
                                        nc.tensor.matmul(
                                            out=sx[h][:, off:off + 128],
                                            lhsT=ident_sb,
                                            rhs=ltri_sb,
                                            start=False, stop=True,
                                        )
                                for h in heads:
                                    px = p_pool.tile([128, 1024], D_Y,
                                                     tag="psb",
                                                     name=f"psb_{h}_{j}_{i}")
                                    nc.scalar.activation(
                                        out=px[:, off:], in_=sx[h][:, off:],
                                        func=mybir.ActivationFunctionType.Exp,
                                    )
                                    pend[h].append((i, px, ch))

                            def emit_y(h):
                                i, px, ch = pend[h].pop(0)
                                for (lo, hi) in reversed(ch):
                                    stop_i = 8 * j + (3 if lo < 512 else 7)
                                    base_v = (i * HPC + h) * VW
                                    nc.tensor.matmul(
                                        out=yps[h][:, lo:hi],
                                        lhsT=vext_sb[:, base_v:base_v + VW],
                                        rhs=px[:, lo:hi],
                                        start=(i == 0), stop=(i == stop_i),
                                    )

                            emit_s2(0)
                            for t in deferred:
                                t()
                            deferred = []
                            for i in range(1, n_i):
                                emit_s2(i)
                                for h in heads:
                                    emit_y(h)
                            for h in heads:
                                emit_y(h)
                            deferred = [
                                (lambda yt=yps[h], hh=h, jj=j:
                                 norm(yt, hh, jj))
                                for h in heads
                            ]
                    for t in deferred:
                        t()

                # ---------------- output projection ----------------
                with (
                    tc.tile_pool(name="osb_pool", bufs=4) as osb_pool,
                    tc.tile_pool(name="o_ps_pool", bufs=4,
                                 space="PSUM") as o_ps_pool,
                ):
                    if True:
                        for tt in range(n_tt):
                            ops = [o_ps_pool.tile([128, 512], F32, tag="ops",
                                                  name=f"ops_{tt}_{cs}")
                                   for cs in range(2)]
                            for k in range(2):
                                for cs in range(2):
                                    nc.tensor.matmul(
                                        out=ops[cs][:],
                                        lhsT=yT[k][:, tt * 128:(tt + 1) * 128],
                                        rhs=wo_sb[k][:, cs * 512:(cs + 1) * 512],
                                        start=(k == 0), stop=(k == 1),
                                    )
                            for cs in range(2):
                                osb = osb_pool.tile([128, 512], D_P,
                                                    tag="osb",
                                                    name=f"osb_{tt}_{cs}")
                                # split the PSUM->SBUF copies between scalar
                                # (idle after exp) and vector
                                if (tt + cs) % 2 == 0:
                                    nc.scalar.copy(out=osb, in_=ops[cs][:])
                                else:
                                    nc.vector.tensor_copy(out=osb,
                                                          in_=ops[cs][:])
                                dq = nc.sync if cs == 0 else nc.gpsimd
                                dq.dma_start(
                                    out=out[tt * 128:(tt + 1) * 128,
                                            cs * 512:(cs + 1) * 512],
                                    in_=osb,
                                )
                rope_cm.__exit__(None, None, None)
    nc.compile()
    names = dict(
        xT=xT.name, wqk=wqk.name, wv=wv.name, wo=wo.name,
        cost=cost.name, ssin=ssin.name, utri=utri.name, ident=ident.name,
        ones4=ones4.name, out=out.name,
    )
    return nc, names


# Head-dim permutation: evens first, odds last — turns the interleaved
# rotate-half pair swap into a contiguous 32-row block swap on device.
PERM = np.concatenate([np.arange(0, HD, 2), np.arange(1, HD, 2)])


def _host_constants(t_len=T, dtcfg=None):
    dtcfg = dtcfg or DTCFG
    inv_freq = 1.0 / (ROPE_BASE ** (np.arange(0, HD, 2, dtype=np.float64) / HD))
    t = np.arange(t_len, dtype=np.float64)
    freqs = np.outer(t, inv_freq)                      # [T, 32]
    emb = np.concatenate([freqs, freqs], axis=-1)      # [T, 64]
    cosT = np.cos(emb).T.astype(np.float32)            # [64, T]
    sinT = np.sin(emb).T.astype(np.float32)
    sgn = np.where(np.arange(HD) % 2 == 0, -1.0, 1.0).astype(np.float32)
    ssinT = sinT * sgn[:, None]
    cosP, ssinP = cosT[PERM], ssinT[PERM]
    d_s = _np_dt(dtcfg[1])
    cos128 = np.vstack([cosP, cosP]).astype(d_s)       # [128, T] bf16
    ssin128 = np.vstack([ssinP, ssinP]).astype(d_s)
    d_y = _np_dt(dtcfg[2])
    # strict-lower-triangular additive mask: exp(score - 1e4) underflows to 0
    ltri = (np.tril(np.full((128, 128), -1.0e4, dtype=np.float32), -1)
            ).astype(d_y)
    ident = np.eye(128, dtype=np.float32).astype(d_y)
    # per head-tile: col 64 = ones (softmax denominator), cols 65-127 = zero
    o4 = np.zeros((128, (t_len // 128) * HPC, VW - HD), dtype=np.float32)
    o4[:, :, 0] = 1.0
    ones4 = o4.reshape(128, -1).astype(d_y)
    return cos128, ssin128, ltri, ident, ones4


def _perm_heads(w):
    Cdim = w.shape[0]
    return w.reshape(Cdim, HPC, HD)[:, :, PERM].reshape(Cdim, HPC * HD)


def _core_inputs(x, w_attn, w_proj, t_len=T, dtcfg=None):
    dtcfg = dtcfg or DTCFG
    d_qkv, d_p = _np_dt(dtcfg[0]), _np_dt(dtcfg[3])
    cos128, ssin128, ltri, ident, ones4 = _host_constants(t_len, dtcfg)
    per_core = []
    for core in range(N_CORES):
        b, g = divmod(core, 4)
        h0 = g * HPC * HD
        wq = _perm_heads(w_attn[:, h0:h0 + HPC * HD])
        wk = _perm_heads(w_attn[:, C + h0:C + h0 + HPC * HD]
                         * np.float32(1.0 / np.sqrt(HD)))
        wvs = w_attn[:, 2 * C + h0:2 * C + h0 + HPC * HD]
        per_core.append(dict(
            xT=np.ascontiguousarray(x[b].T).astype(d_qkv),
            wqk=np.ascontiguousarray(np.concatenate([wq, wk], axis=1)).astype(d_qkv),
            wv=np.ascontiguousarray(wvs).astype(d_qkv),
            wo=np.ascontiguousarray(w_proj[h0:h0 + HPC * HD, :]).astype(d_p),
            cost=cos128, ssin=ssin128, utri=ltri, ident=ident, ones4=ones4,
        ))
    return per_core


def kernel(x, w_attn, w_proj):
    from concourse.bass_utils import run_bass_kernel_spmd

    x = np.asarray(x, dtype=np.float32)
    w_attn = np.asarray(w_attn, dtype=np.float32)
    w_proj = np.asarray(w_proj, dtype=np.float32)

    if "nc" not in _CACHE:
        _CACHE["nc"], _CACHE["names"] = _build(T)
    nc, names = _CACHE["nc"], _CACHE["names"]

    per_core = _core_inputs(x, w_attn, w_proj, T)
    in_maps = [{names[k]: v for k, v in m.items()} for m in per_core]
    r = run_bass_kernel_spmd(nc, in_maps, core_ids=list(range(N_CORES)))

    full = np.zeros((B, T, C), dtype=np.float64)
    for core in range(N_CORES):
        full[core // 4] += r.results[core][names["out"]].astype(np.float64)
    return full.astype(np.float32)
